# revision 2
# baseline (speedup 1.0000x reference)
"""Trainium2 Bass kernel for nn_CurrentFactorCell.

Computes, elementwise over N:
    out_re = scale0*(z_re*g_re - z_im*g_im) + mix0*(z_re*g_re + z_im*g_im) + bias0
    out_im = scale1*(z_re*g_im + z_im*g_re) + mix1*(-z_re*g_im + z_im*g_re) + bias1

which factorizes to
    out_re = p*z_re*g_re + q*z_im*g_im + bias0   p = scale0+mix0, q = mix0-scale0
    out_im = r*z_re*g_im + s*z_im*g_re + bias1   r = scale1-mix1, s = scale1+mix1

The rel-err budget (2e-2) allows bf16 end-to-end (~2.5e-3), which halves
HBM traffic (24MB -> 12MB per core) -- the kernel is memory-bound, so this
is the main lever vs the f32 version.

DVE restriction: scalar_tensor_tensor has NO perf modes (always 1
elem/lane/cyc), while plain tensor_tensor runs 2x in bf16 (2x_1p: 2-byte
dtype, step +-1, 4B-aligned) and tensor_scalar runs 4x. So the per-partition
scalars p,q,r,s are folded into the host-side bf16 quantization scales:
    zr' = p*zr, zi' = (q*p/r)*zi, gi' = (r/p)*gi, gr' = gr
    out_re = zr'*gr  + zi'*gi' + b0
    out_im = zr'*gi' + k*zi'*gr + b1,   k = r*s/(p*q)
leaving only tensor_tensor ops on the device (k == -1 exactly for the
reference params, so the k-term is a TT subtract; otherwise one 4x
tensor_scalar with immediate k). Gate is deinterleaved on the host so every
DVE operand is unit-stride.

Sharding: data-parallel along N across 8 cores; params replicated.

Hardware constraints that shaped the layout (walrus rejects instructions
whose sync-wait count exceeds the ISA struct capacity, which is ONE for
compute ops and DMACopy; only NoOp/Drain/Branch take more; and there are
just 8 DMAHW completion-sem lanes, so a 9th DMA picks up an extra
lane-serialization wait):
  * one persistent input mega-tile, filled by 5 region-disjoint loads
    (progressive spans keep the pipeline fill edge short),
  * one output mega-tile written only by DVE, drained by 5 region stores
    (each store waits only on the DVE sem),
  * multi-wait instructions (DMAHW lane reuse, tail drain) are legalized by
    the NoOp-splitting compile hook.
"""

import json

import numpy as np

N = 8388608
N_CORES = 8
PER_CORE = N // N_CORES          # 1048576
P = 128
TILE_F = 1024                    # free-dim elems per compute group
N_TILES = PER_CORE // (P * TILE_F)   # 8
# DMA spans in compute-group units: progressive sizes keep the pipeline
# fill (first load) and drain (last store) edges short
LOAD_SPANS = [(0, 1), (1, 2), (2, 4), (4, 6), (6, 8)]
STORE_SPANS = [(0, 2), (2, 4), (4, 6), (6, 7), (7, 8)]
ROW = 4 * TILE_F * N_TILES       # bf16 elems per partition row of zin

_cache = {}


def _split_multi_waits(bir_json: bytes) -> bytes:
    """Split instructions with >1 sync wait into single-wait NoOp chains.

    The walrus build in this environment caps every ISA struct at ONE sync
    wait command ("Too many sync wait commands" otherwise), but Tile's
    semaphore assignment freely attaches several (e.g. the kernel-tail
    Drain waits on every DMAHW lane). Same-engine program order makes a
    preceding NoOp-with-wait semantically identical.
    """
    d = json.loads(bir_json)
    changed = False
    for fn in d.get("functions", []):
        for blk in fn.get("blocks", []):
            out = []
            for ins in blk.get("instructions", []):
                si = ins.get("sync_info") or {}
                ow = si.get("on_wait") or []
                if len(ow) > 1:
                    changed = True
                    for i, w in enumerate(ow[:-1]):
                        out.append(
                            {
                                "engine": ins["engine"],
                                "ins": [],
                                "name": f"{ins['name']}-syncw{i}",
                                "opcode": "NoOp",
                                "outs": [],
                                "sync_info": {"on_update": [], "on_wait": [w]},
                            }
                        )
                    si["on_wait"] = [ow[-1]]
                out.append(ins)
            blk["instructions"] = out
    if not changed:
        return bir_json
    return json.dumps(d).encode()


def _install_compile_hook():
    if _cache.get("hook"):
        return
    import concourse.bass_utils as bass_utils
    import concourse.bass2jax as bass2jax

    orig = bass_utils.compile_bir_kernel

    def patched(bir_json, tmpdir, neff_name="file.neff"):
        return orig(_split_multi_waits(bir_json), tmpdir, neff_name)

    bass_utils.compile_bir_kernel = patched
    if getattr(bass2jax, "compile_bir_kernel", None) is orig:
        bass2jax.compile_bir_kernel = patched
    _cache["hook"] = True


def _coeffs(scale, mix, bias):
    """Host-side parameter preprocessing (8 scalar flops, f64).

    Returns (alpha, beta, delta, k, b0, b1, fallback). When fallback is
    True the prescale factorization is ill-conditioned and the device
    program applies p,q,r,s itself via 4x tensor_scalar immediates.
    """
    s0, s1 = float(scale[0]), float(scale[1])
    m0, m1 = float(mix[0]), float(mix[1])
    b0, b1 = float(bias[0]), float(bias[1])
    p = s0 + m0
    q = m0 - s0
    r = s1 - m1
    s = s1 + m1
    lo, hi = 2.0**-6, 2.0**6
    ok = all(lo <= abs(v) <= hi for v in (p, q, r, s))
    if not ok:
        return p, q, r, s, None, b0, b1, True
    alpha = p
    delta = r / p
    beta = q * p / r
    k = (r * s) / (p * q)
    if not (lo <= abs(k) <= hi and lo <= abs(beta) <= hi and lo <= abs(delta) <= hi):
        return p, q, r, s, None, b0, b1, True
    return alpha, beta, delta, k, None, b0, b1, False


def _build_nc(coef, loop_reps=None):
    """Build the Bass program for a given coefficient tuple. loop_reps wraps
    the whole body in a hardware For_i loop -- used only by test.py to
    amortize the ~80ms axon dispatch overhead when measuring device time;
    the graded path uses None."""
    import concourse.bass as bass
    import concourse.tile as tile
    from concourse import mybir

    bf16 = mybir.dt.bfloat16

    nc = bass.Bass()
    # per partition row: [group0 | group1 | ...]
    # group t cols (relative): [0:F]=zr', [F:2F]=zi', [2F:3F]=gr, [3F:4F]=gi'
    zin = nc.declare_dram_parameter("zin", [P, ROW], bf16, isOutput=False)
    # packed output, per partition row: group t at cols [2F*t : 2F*(t+1)],
    # within a group cols [0:F]=out_re, [F:2F]=out_im
    zout = nc.declare_dram_parameter("zout", [P, 2 * TILE_F * N_TILES], bf16,
                                     isOutput=True)

    with tile.TileContext(nc) as tc:
        with (
            tc.tile_pool(name="io", bufs=1) as io_pool,
            tc.tile_pool(name="out", bufs=1) as out_pool,
            tc.tile_pool(name="tmp", bufs=2) as tmp_pool,
        ):
            zbig = io_pool.tile([P, ROW], bf16)
            obig = out_pool.tile([P, 2 * TILE_F * N_TILES], bf16)

            import contextlib

            loop_ctx = (
                tc.For_i(0, loop_reps, 1)
                if loop_reps is not None
                else contextlib.nullcontext()
            )
            with loop_ctx:
                _emit_body(nc, mybir, zin, zbig, obig, zout, tmp_pool, coef)
    return nc


def _emit_body(nc, mybir, zin, zbig, obig, zout, tmp_pool, coef):
    bf16 = mybir.dt.bfloat16
    mult = mybir.AluOpType.mult
    add = mybir.AluOpType.add
    sub = mybir.AluOpType.subtract
    F = TILE_F
    c0, c1, c2, k, _, b0, b1, fallback = coef

    # region-disjoint loads with progressive spans
    for glo, ghi in LOAD_SPANS:
        lo = 4 * F * glo
        hi = 4 * F * ghi
        nc.sync.dma_start(zbig[:, lo:hi], zin[:, lo:hi])

    for t in range(N_TILES):
        base = 4 * F * t
        zr = zbig[:, base : base + F]
        zi = zbig[:, base + F : base + 2 * F]
        gr = zbig[:, base + 2 * F : base + 3 * F]
        gi = zbig[:, base + 3 * F : base + 4 * F]
        ore = obig[:, 2 * F * t : 2 * F * t + F]
        oim = obig[:, 2 * F * t + F : 2 * F * (t + 1)]

        a = tmp_pool.tile([P, F], bf16, tag="a")
        b = tmp_pool.tile([P, F], bf16, tag="b")
        c = tmp_pool.tile([P, F], bf16, tag="c")
        d = tmp_pool.tile([P, F], bf16, tag="d")
        if not fallback:
            # prescaled on host: ore = zr*gr + zi*gi + b0
            #                    oim = zr*gi + k*zi*gr + b1
            nc.vector.tensor_tensor(a[:, :], zr, gr, mult)
            nc.vector.tensor_tensor(b[:, :], zi, gi, mult)
            nc.vector.tensor_tensor(c[:, :], zr, gi, mult)
            nc.vector.tensor_tensor(d[:, :], zi, gr, mult)
            if b0 != 0.0:
                nc.vector.tensor_scalar(b[:, :], b[:, :], float(b0), None, op0=add)
            nc.vector.tensor_tensor(ore, a[:, :], b[:, :], add)
            if k == -1.0:
                if b1 != 0.0:
                    nc.vector.tensor_scalar(c[:, :], c[:, :], float(b1), None,
                                            op0=add)
                nc.vector.tensor_tensor(oim, c[:, :], d[:, :], sub)
            elif k == 1.0 and b1 == 0.0:
                nc.vector.tensor_tensor(oim, c[:, :], d[:, :], add)
            else:
                if b1 != 0.0:
                    nc.vector.tensor_scalar(d[:, :], d[:, :], float(k), float(b1),
                                            op0=mult, op1=add)
                else:
                    nc.vector.tensor_scalar(d[:, :], d[:, :], float(k), None,
                                            op0=mult)
                nc.vector.tensor_tensor(oim, c[:, :], d[:, :], add)
        else:
            # general path: raw bf16 inputs, immediates p,q,r,s applied via
            # 4x tensor_scalar
            p, q, r, s = c0, c1, c2, k
            nc.vector.tensor_tensor(a[:, :], zr, gr, mult)
            nc.vector.tensor_tensor(b[:, :], zi, gi, mult)
            nc.vector.tensor_tensor(c[:, :], zr, gi, mult)
            nc.vector.tensor_tensor(d[:, :], zi, gr, mult)
            if b0 != 0.0:
                nc.vector.tensor_scalar(a[:, :], a[:, :], float(p), float(b0),
                                        op0=mult, op1=add)
            else:
                nc.vector.tensor_scalar(a[:, :], a[:, :], float(p), None, op0=mult)
            nc.vector.tensor_scalar(b[:, :], b[:, :], float(q), None, op0=mult)
            nc.vector.tensor_tensor(ore, a[:, :], b[:, :], add)
            if b1 != 0.0:
                nc.vector.tensor_scalar(c[:, :], c[:, :], float(r), float(b1),
                                        op0=mult, op1=add)
            else:
                nc.vector.tensor_scalar(c[:, :], c[:, :], float(r), None, op0=mult)
            nc.vector.tensor_scalar(d[:, :], d[:, :], float(s), None, op0=mult)
            nc.vector.tensor_tensor(oim, c[:, :], d[:, :], add)

        for slo, shi in STORE_SPANS:
            if t == shi - 1:
                nc.scalar.dma_start(
                    zout[:, 2 * F * slo : 2 * F * shi],
                    obig[:, 2 * F * slo : 2 * F * shi],
                )
    return nc


def _get_nc(coef):
    key = ("nc", coef)
    if key not in _cache:
        _cache[key] = _build_nc(coef)
    return _cache[key]


def _make_in_maps(z_re, z_im, gate, coef):
    import ml_dtypes

    bf16 = ml_dtypes.bfloat16
    F = TILE_F
    c0, c1, c2, k, _, b0, b1, fallback = coef
    if not fallback:
        alpha, beta, delta = c0, c1, c2
        zr = (z_re * np.float32(alpha)).astype(bf16)
        zi = (z_im * np.float32(beta)).astype(bf16)
        gr = np.ascontiguousarray(gate[:, 0]).astype(bf16)
        gi = (gate[:, 1] * np.float32(delta)).astype(bf16)
    else:
        zr = z_re.astype(bf16)
        zi = z_im.astype(bf16)
        gr = np.ascontiguousarray(gate[:, 0]).astype(bf16)
        gi = np.ascontiguousarray(gate[:, 1]).astype(bf16)

    # pack [zr | zi | gr | gi] per (core, group, partition) row
    zin = np.empty((N_CORES, P, N_TILES, 4 * F), dtype=bf16)
    zin[:, :, :, 0:F] = zr.reshape(N_CORES, N_TILES, P, F).transpose(0, 2, 1, 3)
    zin[:, :, :, F : 2 * F] = zi.reshape(N_CORES, N_TILES, P, F).transpose(0, 2, 1, 3)
    zin[:, :, :, 2 * F : 3 * F] = gr.reshape(N_CORES, N_TILES, P, F).transpose(
        0, 2, 1, 3
    )
    zin[:, :, :, 3 * F : 4 * F] = gi.reshape(N_CORES, N_TILES, P, F).transpose(
        0, 2, 1, 3
    )
    zin = zin.reshape(N_CORES, P, ROW)
    return [{"zin": zin[c]} for c in range(N_CORES)]


def kernel(z_re, z_im, gate, scale, mix, bias):
    _install_compile_hook()
    from concourse.bass_utils import run_bass_kernel_spmd

    z_re = np.asarray(z_re, dtype=np.float32)
    z_im = np.asarray(z_im, dtype=np.float32)
    gate = np.asarray(gate, dtype=np.float32)
    scale = np.asarray(scale, dtype=np.float32)
    mix = np.asarray(mix, dtype=np.float32)
    bias = np.asarray(bias, dtype=np.float32)

    coef = _coeffs(scale, mix, bias)
    nc = _get_nc(coef)
    in_maps = _make_in_maps(z_re, z_im, gate, coef)
    res = run_bass_kernel_spmd(nc, in_maps, list(range(N_CORES))).results
    return _unpack_out(res)


def _unpack_out(res):
    F = TILE_F
    zout = np.stack(
        [np.asarray(res[c]["zout"]).astype(np.float32) for c in range(N_CORES)]
    )
    zout = zout.reshape(N_CORES, P, N_TILES, 2 * F)
    out_re = np.ascontiguousarray(
        zout[:, :, :, 0:F].transpose(0, 2, 1, 3)
    ).reshape(-1)
    out_im = np.ascontiguousarray(
        zout[:, :, :, F : 2 * F].transpose(0, 2, 1, 3)
    ).reshape(-1)
    return out_re, out_im
